# revision 2
# baseline (speedup 1.0000x reference)
"""Trainium2 Bass kernel: batched crop + bilinear resize (nn_Cropping).

Full inputs: x [8, 3, 1024, 1024] f32, bbox [128, 4] f32 (normalized cxcywh).
Full output: [128, 8, 3, 50, 50] f32.

Strategy: data-parallel over batch B=8 across the 8 NeuronCores (core b owns
image b). Bilinear crop-resize per box is expressed as two sparse
interpolation-matrix matmuls on the PE:

    S_T = IMG_window^T @ A_n      (y-interp; image window is the stationary
                                   operand so the intermediate comes out
                                   already transposed: [w_chunk, 50])
    out = S_T^T @ B_n             (x-interp; [50, 50] per box/channel)

A_n [region_rows, 50] and B_n [w_n, 50] hold the bilinear gather weights
(2 nonzeros per column) and are precomputed on the host from bbox — index
math only; all pixel arithmetic runs on device. The kernel program is
JIT-specialized on the bbox values (crop windows become static AP slices).
"""

import os
import sys
import types
import numpy as np

OUT = 50
H = 1024
W = 1024
C = 3
N_BOXES = 128
N_CORES = 8
P = 128

_DT = None  # lazy: mybir.dt


def _xyxy_int(bbox):
    """Mirror reference._xyxy_int in strict float32 numpy."""
    scale = np.array([W, H, W, H], dtype=np.float32)
    b = (bbox.astype(np.float32) * scale).astype(np.float32)
    cx, cy, w, h = b[:, 0], b[:, 1], b[:, 2], b[:, 3]
    x1 = np.clip(np.floor(cx - w / np.float32(2)).astype(np.int32), 0, W - 1)
    y1 = np.clip(np.floor(cy - h / np.float32(2)).astype(np.int32), 0, H - 1)
    x2 = np.clip(np.floor(cx + w / np.float32(2)).astype(np.int32), 0, W)
    y2 = np.clip(np.floor(cy + h / np.float32(2)).astype(np.int32), 0, H)
    x2 = np.maximum(x2, x1 + 1)
    y2 = np.maximum(y2, y1 + 1)
    return x1, y1, x2, y2


def _src_coords(lo, hi):
    """Mirror reference._src_coords in strict float32 numpy (scalar lo/hi)."""
    n = np.float32(hi - lo)
    j = np.arange(OUT, dtype=np.float32)
    s = np.clip((j + np.float32(0.5)) * n / np.float32(OUT) - np.float32(0.5),
                np.float32(0.0), n - np.float32(1.0)).astype(np.float32)
    i0 = np.floor(s)
    w1 = (s - i0).astype(np.float32)
    i0 = i0.astype(np.int32)
    i1 = np.minimum(i0 + 1, hi - lo - 1)
    return lo + i0, lo + i1, w1


def _build_plan(bbox):
    """Host-side index/weight precompute. Returns a dict with everything the
    program builder needs plus the A/B concatenated weight arrays."""
    x1, y1, x2, y2 = _xyxy_int(bbox)
    n = bbox.shape[0]

    ylo = int(y1.min())
    yhi = int(y2.max())
    xlo = int(x1.min())
    xhi = int(x2.max())
    n_row_tiles = (yhi - ylo + P - 1) // P
    # Region rows actually loaded: ylo .. ylo + n_row_tiles*P (clipped to H).
    w_r = xhi - xlo

    boxes = []
    a_chunks = []  # list of [P, OUT] fp32
    b_chunks = []  # list of [P, OUT] fp32
    for i in range(n):
        gy0, gy1, wy = _src_coords(int(y1[i]), int(y2[i]))
        gx0, gx1, wx = _src_coords(int(x1[i]), int(x2[i]))
        h_i = int(y2[i] - y1[i])
        w_i = int(x2[i] - x1[i])

        # A: [region_rows, OUT] sparse, grid-aligned to P-row tiles of region.
        t_lo = (int(y1[i]) - ylo) // P
        t_hi = (int(y2[i]) - 1 - ylo) // P
        row_tiles = list(range(t_lo, t_hi + 1))
        a_full = np.zeros((n_row_tiles * P, OUT), dtype=np.float32)
        np.add.at(a_full, (gy0 - ylo, np.arange(OUT)), 1.0 - wy)
        np.add.at(a_full, (gy1 - ylo, np.arange(OUT)), wy)
        a_idx = []
        for t in row_tiles:
            a_idx.append(len(a_chunks))
            a_chunks.append(a_full[t * P:(t + 1) * P, :])

        # B: [w_i, OUT] sparse, window-aligned; split into P-row chunks.
        b_full = np.zeros((w_i, OUT), dtype=np.float32)
        np.add.at(b_full, (gx0 - x1[i], np.arange(OUT)), 1.0 - wx)
        np.add.at(b_full, (gx1 - x1[i], np.arange(OUT)), wx)
        n_cx = (w_i + P - 1) // P
        b_idx = []
        cc_list = []
        for ci in range(n_cx):
            cc = min(P, w_i - ci * P)
            chunk = np.zeros((P, OUT), dtype=np.float32)
            chunk[:cc, :] = b_full[ci * P:ci * P + cc, :]
            b_idx.append(len(b_chunks))
            b_chunks.append(chunk)
            cc_list.append(cc)

        boxes.append(dict(
            xoff=int(x1[i]) - xlo,
            w=w_i, h=h_i,
            row_tiles=row_tiles, a_idx=a_idx,
            n_cx=n_cx, b_idx=b_idx, cc=cc_list,
        ))

    a_cat = np.concatenate(a_chunks, axis=1).astype(np.float16)  # [P, nA*OUT]
    b_cat = np.concatenate(b_chunks, axis=1).astype(np.float16)  # [P, nB*OUT]
    return dict(
        ylo=ylo, xlo=xlo, w_r=w_r, n_row_tiles=n_row_tiles,
        boxes=boxes, a_cat=a_cat, b_cat=b_cat,
    )


def _install_tile_patch(tile_mod):
    """TileContext that never leaves more than one sem wait on any lowered
    instruction (the walrus in this toolchain rejects multi-wait sync fields
    on several instruction structs, e.g. Matmult and Drain). Excess waits are
    re-emitted as standalone wait_ge instructions on the same engine right
    before the instruction, which is sync-equivalent."""
    from concourse.vector_clock import ScopedClock

    class PatchedTileContext(tile_mod.TileContext):
        _MAX_WAITS = 1

        def _split_excess_waits(self, inst):
            si = getattr(inst, "sync_info", None)
            if si is None:
                return
            waits = list(si.on_wait)
            if len(waits) <= self._MAX_WAITS:
                return
            id2sem = {s.num: s for s in self.sems.allocated().values()}
            eng = self.nc.engines[inst.engine]
            for wt in waits[self._MAX_WAITS:]:
                assert wt.wait_mode == "sem-ge-imm", wt
                eng.wait_ge(id2sem[wt.id], wt.wait_value)
            si.on_wait = waits[:self._MAX_WAITS]

        def _commit_and_lower(self, inst, *args, **kwargs):
            self._split_excess_waits(inst)
            return super()._commit_and_lower(inst, *args, **kwargs)

        def _commit_instruction(self, inst, *args, **kwargs):
            # Loop/critical lowering paths commit without _commit_and_lower;
            # splitting here too keeps every committed instruction <= 1 wait
            # (idempotent if both hooks fire for the same instruction).
            self._split_excess_waits(inst)
            return super()._commit_instruction(inst, *args, **kwargs)

        def _drain_and_barrier(self, tick_clock, wait_clock):
            nc = self.nc
            drain_inst = nc.sync.drain()
            wait_clock.add_sem_waits(
                drain_inst.ins, ScopedClock({None: tick_clock.global_clock}))
            si = drain_inst.ins.sync_info
            waits = list(si.on_wait) if si is not None else []
            if len(waits) > 1:
                si.on_wait = waits[:1]
                id2sem = {s.num: s for s in self.sems.allocated().values()}
                for wt in waits[1:]:
                    nc.sync.wait_ge(id2sem[wt.id], wt.wait_value)
            nc.all_engine_barrier()
            popped = nc._tile_sem_poison_stack.pop()
            assert popped is self._sem_poison
            nc.clear_and_free_semaphores(list(self.sems.allocated().values()))
            nc.all_engine_barrier()

    return PatchedTileContext


def _build_program(plan, repeat=1):
    from contextlib import ExitStack
    import concourse.bass as bass
    import concourse.tile as tile
    from concourse import mybir

    f32 = mybir.dt.float32
    f16 = mybir.dt.float16

    na = plan["a_cat"].shape[1]
    nb = plan["b_cat"].shape[1]
    w_r = plan["w_r"]
    n_rt = plan["n_row_tiles"]
    ylo = plan["ylo"]
    xlo = plan["xlo"]

    nc = bass.Bass("TRN2", target_bir_lowering=False, debug=False,
                   num_devices=1)
    img = nc.dram_tensor("img", [C, H, W], f32, kind="ExternalInput").ap()
    a_in = nc.dram_tensor("a_cat", [P, na], f16, kind="ExternalInput").ap()
    b_in = nc.dram_tensor("b_cat", [P, nb], f16, kind="ExternalInput").ap()
    # Output laid out [i, n, c, j] so per-box staging DMAs write 600B runs.
    out = nc.dram_tensor("out", [OUT, N_BOXES, C, OUT], f32,
                         kind="ExternalOutput").ap()

    TC = _install_tile_patch(tile)
    GROUP = 16  # boxes per output DMA
    max_ncx = max(b["n_cx"] for b in plan["boxes"])
    assert max_ncx * OUT <= 512, "crop too wide for one PSUM bank"
    st_free = max_ncx * OUT

    with TC(nc) as tc:
        with ExitStack() as es:
            if repeat > 1:
                es.enter_context(tc.For_i(0, repeat, 1))
            _emit_body(nc, tc, plan, img, a_in, b_in, out)
    return nc


def _emit_body(nc, tc, plan, img, a_in, b_in, out):
    from concourse import mybir

    f32 = mybir.dt.float32
    f16 = mybir.dt.float16
    na = plan["a_cat"].shape[1]
    nb = plan["b_cat"].shape[1]
    w_r = plan["w_r"]
    n_rt = plan["n_row_tiles"]
    ylo = plan["ylo"]
    xlo = plan["xlo"]

    GROUP = 16  # boxes per output DMA
    max_ncx = max(b["n_cx"] for b in plan["boxes"])
    assert max_ncx * OUT <= 512, "crop too wide for one PSUM bank"
    st_free = max_ncx * OUT

    if True:
        with (
            tc.tile_pool(name="const", bufs=1) as const_pool,
            tc.tile_pool(name="psum_st", bufs=4, space="PSUM") as psum_st_pool,
            tc.tile_pool(name="psum_out", bufs=2, space="PSUM") as psum_out_pool,
            tc.tile_pool(name="st", bufs=6) as st_pool,
            tc.tile_pool(name="staging", bufs=2) as staging_pool,
        ):
            # --- load interp matrices ---
            a_sb = const_pool.tile([P, na], f16, tag="a_sb")
            b_sb = const_pool.tile([P, nb], f16, tag="b_sb")
            nc.sync.dma_start(out=a_sb, in_=a_in)
            nc.sync.dma_start(out=b_sb, in_=b_in)

            # --- load image region, fp32 -> fp16 cast in DMA ---
            region = [[None] * n_rt for _ in range(C)]
            for ch in range(C):
                for t in range(n_rt):
                    r0 = ylo + t * P
                    rows = min(P, H - r0)
                    rt = const_pool.tile([P, w_r], f16, tag=f"reg{ch}_{t}")
                    if rows < P:
                        # Uninitialized SBUF can hold NaN fp16 patterns;
                        # NaN * 0 would poison the matmul accumulation.
                        nc.any.memset(rt[rows:, :], 0)
                    nc.gpsimd.dma_start(
                        out=rt[:rows, :],
                        in_=img[ch, r0:r0 + rows, xlo:xlo + w_r])
                    region[ch][t] = rt

            staging = None
            for bi, box in enumerate(plan["boxes"]):
                g_pos = bi % GROUP
                if g_pos == 0:
                    staging = staging_pool.tile([OUT, GROUP * C * OUT], f32,
                                                tag="staging")

                n_cx = box["n_cx"]
                xoff = box["xoff"]
                st_tiles = []
                for ch in range(C):
                    # Stage A: S_T chunks for all col-chunks of this channel
                    # accumulate into one PSUM tile [P, n_cx*OUT].
                    ps = psum_st_pool.tile([P, st_free], f32, tag="ps_st")
                    for ci in range(n_cx):
                        cc = box["cc"][ci]
                        co = xoff + ci * P
                        n_t = len(box["row_tiles"])
                        for k, t in enumerate(box["row_tiles"]):
                            nc.tensor.matmul(
                                ps[:cc, ci * OUT:(ci + 1) * OUT],
                                lhsT=region[ch][t][:, co:co + cc],
                                rhs=a_sb[:, box["a_idx"][k] * OUT:
                                         (box["a_idx"][k] + 1) * OUT],
                                start=(k == 0), stop=(k == n_t - 1))
                    st = st_pool.tile([P, st_free], f16, tag="st_sb")
                    nc.any.tensor_copy(st[:, :n_cx * OUT], ps[:, :n_cx * OUT])
                    st_tiles.append(st)

                # Stage B: out[50i, 50j] per channel into one PSUM tile.
                po = psum_out_pool.tile([OUT, C * OUT], f32, tag="ps_out")
                for ch in range(C):
                    for ci in range(n_cx):
                        cc = box["cc"][ci]
                        nc.tensor.matmul(
                            po[:, ch * OUT:(ch + 1) * OUT],
                            lhsT=st_tiles[ch][:cc, ci * OUT:(ci + 1) * OUT],
                            rhs=b_sb[:cc, box["b_idx"][ci] * OUT:
                                     (box["b_idx"][ci] + 1) * OUT],
                            start=(ci == 0), stop=(ci == n_cx - 1))
                nc.any.tensor_copy(
                    staging[:, g_pos * C * OUT:(g_pos + 1) * C * OUT], po)

                if g_pos == GROUP - 1 or bi == len(plan["boxes"]) - 1:
                    g0 = bi - g_pos
                    nc.sync.dma_start(
                        out=out[:, g0:bi + 1, :, :],
                        in_=staging[:, :(g_pos + 1) * C * OUT])


LAST_EXEC_NS = None
LAST_TRACE = None
LAST_PROFILE_JSON = None


def kernel(x: np.ndarray, bbox: np.ndarray) -> np.ndarray:
    global LAST_EXEC_NS, LAST_TRACE, LAST_PROFILE_JSON
    from concourse import bass_utils

    x = np.asarray(x, dtype=np.float32)
    bbox = np.asarray(bbox, dtype=np.float32)
    plan = _build_plan(bbox)
    nc = _build_program(plan)

    in_maps = [
        {"img": np.ascontiguousarray(x[b]),
         "a_cat": plan["a_cat"],
         "b_cat": plan["b_cat"]}
        for b in range(N_CORES)
    ]
    res = bass_utils.run_bass_kernel_spmd(nc, in_maps,
                                          core_ids=list(range(N_CORES)))
    LAST_EXEC_NS = res.exec_time_ns
    if res.instructions_and_trace is not None:
        LAST_TRACE = res.instructions_and_trace[1]
    LAST_PROFILE_JSON = res.profile_json
    # res.results[b]["out"]: [OUT, N_BOXES, C, OUT] -> [N_BOXES, C, OUT, OUT]
    full = np.empty((N_BOXES, N_CORES, C, OUT, OUT), dtype=np.float32)
    for b in range(N_CORES):
        full[:, b] = res.results[b]["out"].transpose(1, 2, 0, 3)
    return full


if __name__ == "__main__":
    rng = np.random.default_rng(0)
    xs = rng.standard_normal((N_CORES, C, H, W), dtype=np.float32)
    u = rng.random((N_BOXES, 4), dtype=np.float32)
    bb = np.stack([0.3 + 0.4 * u[:, 0], 0.3 + 0.4 * u[:, 1],
                   0.1 + 0.2 * u[:, 2], 0.1 + 0.2 * u[:, 3]], axis=-1)
    y = kernel(xs, bb)
    print("out", y.shape, y.dtype, np.abs(y).max())

